# revision 1
# baseline (speedup 1.0000x reference)
"""Trainium2 Bass kernel for ConfidenceGNNFusion (nn_ConfidenceGNNFusion).

Pipeline (per the reference model):
  xe  = relu(BN(conv1x1(x))) * conf
  xp  = relu(BN(conv3x3(relu(BN(conv3x3(xe))))))
  xn  = mean_hw(xp)                         # (N, HID) node features
  xg  = relu(GAT2(relu(GAT1(xn))))          # two 4-head GAT layers
  out = conv1x1(xg[:, :, None, None] + xp)  # == conv1x1(xp) + op_w @ xg

Sharding: data-parallel over images/nodes (64 per core, 8 cores). The
pooled node features ([128 feat, 64 nodes] per core) are exchanged with
an AllGather; both GAT layers run with dst nodes sharded per core (a
second AllGather redistributes layer-1 outputs). conv1x1(xp) ("y1")
stays resident in SBUF between the conv pass and the final bias-add
pass, so the big activations never round-trip HBM.

GAT dense-mask form (exact softmax up to the shift invariance; scores
are O(1) so skipping the max-subtraction cannot overflow):
  exT[s, d] = exp(lrelu(a_s[s] + a_d[d]) + log(cnt[s, d]))
  out[d]    = (1/sum_s exT[s, d]) * sum_s exT[s, d] * h[s]
cnt = host-precomputed edge multiplicity (incl. self loops); absent
edges get log-count -60 -> exp underflows to ~1e-27.

All matmul operands are float32r (TF32-like: measured 1.5e-4 rel err on
HW vs 2.5e-3 for bf16) which streams at bf16 speed for N>=256.
"""
import numpy as np

import concourse.bass as bass  # noqa: F401  (kept for interactive use)
import concourse.mybir as mybir
import concourse.tile as tile
from concourse import bacc
from concourse.bass_utils import run_bass_kernel_spmd

F32 = mybir.dt.float32
F32R = mybir.dt.float32r
AF = mybir.ActivationFunctionType
ALU = mybir.AluOpType
AX = mybir.AxisListType

NCORES = 8
CIN = 64
HID = 128
HW = 1024            # 32*32
HEADS = 4
CANV = 34            # zero-padded canvas edge (32 + 2)
NCBUF = 2            # canvas ring depth
BIG_NEG = -60.0      # log-count for absent edges
BN_EPS = 1e-5


def _chunks(n, step=128):
    return [(s, min(step, n - s)) for s in range(0, n, step)]


def _gat_layer(nc, sb, ps, *, imgs, nodes, xt_all, x_own, g_lin, att_s_bc,
               att_cols, g_b, lc_t, ones_col, out_xg, li):
    """One dense GAT layer; dst nodes = this core's `imgs`.

    xt_all: [128 feat, nodes] f32r, features of ALL nodes (feat-major)
    x_own:  [128 feat, imgs] f32r, features of OWN nodes
    g_lin:  [128, 512] f32r lhsT (in_feat, heads*D)
    att_s_bc: [128, 512] att_s flat row replicated over partitions
    att_cols: [128, 8] f32r; cols 0-3 att_s per head, 4-7 att_d per head
    lc_t:   [128, nch*imgs] log-count^T (src chunk c at cols c*imgs..)
    out_xg: [128, imgs] f32r output (feat-major, own nodes)
    """
    ch = _chunks(nodes)
    nch = len(ch)

    # 1) h node-major for all nodes: th[c] = [node chunk, heads*D]
    th = []
    for c, (s0, sz) in enumerate(ch):
        ph = ps.tile([128, 512], F32, tag="g_h", name="g_h", bufs=2)
        nc.tensor.matmul(ph[:sz, :], xt_all[:, s0:s0 + sz], g_lin[:],
                         start=True, stop=True)
        t = sb.tile([128, 512], F32R, tag=f"g_th{c}_{li}", name=f"g_th{c}_{li}")
        nc.scalar.copy(t[:sz, :], ph[:sz, :])
        th.append(t)

    # 2) a_s node-major: a_s[node, head] = sum_d h * att_s
    ta_s = sb.tile([128, 4 * nch], F32, tag=f"g_as_{li}", name=f"g_as_{li}")
    for c, (s0, sz) in enumerate(ch):
        tm = sb.tile([128, 512], F32, tag="g_astmp", name="g_astmp")
        nc.vector.tensor_mul(tm[:sz, :], th[c][:sz, :], att_s_bc[:sz, :])
        nc.vector.tensor_reduce(
            ta_s[:sz, c * 4:(c + 1) * 4],
            tm[:sz, :].rearrange("p (h d) -> p h d", h=HEADS),
            axis=AX.X, op=ALU.add)

    # 3) a_d of own nodes as a partition-broadcast row, per head:
    #    hT_own = lin_h^T @ x_own ; a_d_own = att_d_h^T @ hT_own
    a_d_bc = []
    for h in range(HEADS):
        pho = ps.tile([128, imgs], F32, tag="g_hto", name="g_hto")
        nc.tensor.matmul(pho[:], g_lin[:, h * 128:(h + 1) * 128], x_own[:],
                         start=True, stop=True)
        tho = sb.tile([128, imgs], F32R, tag="g_hto_s", name="g_hto_s")
        nc.scalar.copy(tho[:], pho[:])
        pad = ps.tile([1, imgs], F32, tag="g_adp", name="g_adp")
        nc.tensor.matmul(pad[:], att_cols[:, 4 + h:5 + h], tho[:],
                         start=True, stop=True)
        tad = sb.tile([1, imgs], F32, tag="g_adr", name="g_adr")
        nc.scalar.copy(tad[:], pad[:])
        tbc = sb.tile([128, imgs], F32, tag=f"g_adbc{h}_{li}", name=f"g_adbc{h}_{li}")
        nc.gpsimd.partition_broadcast(tbc[:], tad[:])
        a_d_bc.append(tbc)

    # 4) exT[src, dst] per (head, chunk) + per-head denominator
    ex = {}
    rec = []
    for h in range(HEADS):
        pden = ps.tile([1, imgs], F32, tag="g_den", name="g_den")
        for c, (s0, sz) in enumerate(ch):
            t = sb.tile([128, imgs], F32R, tag=f"g_ex{h}_{c}_{li}", name=f"g_ex{h}_{c}_{li}")
            # S^T = a_d[dst] + a_s[src]; lrelu; + logcnt; exp
            nc.vector.tensor_scalar(
                out=t[:sz, :], in0=a_d_bc[h][:sz, :],
                scalar1=ta_s[:sz, c * 4 + h:c * 4 + h + 1], scalar2=None,
                op0=ALU.add)
            nc.vector.scalar_tensor_tensor(
                t[:sz, :], t[:sz, :], 0.2, t[:sz, :],
                op0=ALU.mult, op1=ALU.max)
            nc.vector.tensor_add(t[:sz, :], t[:sz, :],
                                 lc_t[:sz, c * imgs:(c + 1) * imgs])
            nc.scalar.activation(t[:sz, :], t[:sz, :], AF.Exp)
            nc.tensor.matmul(pden[:], ones_col[:sz, :], t[:sz, :],
                             start=(c == 0), stop=(c == nch - 1))
            ex[(h, c)] = t
        trec = sb.tile([1, imgs], F32, tag=f"g_rec{h}", name=f"g_rec{h}")
        nc.vector.reciprocal(trec[:], pden[:])
        tbc = sb.tile([128, imgs], F32, tag=f"g_recbc{h}_{li}", name=f"g_recbc{h}_{li}")
        nc.gpsimd.partition_broadcast(tbc[:], trec[:])
        rec.append(tbc)

    # 5) normalize exT, weighted sum over src into one psum, head-mean
    pout = ps.tile([128, imgs], F32, tag="g_out", name="g_out")
    n_mm = 0
    for h in range(HEADS):
        for c, (s0, sz) in enumerate(ch):
            t = ex[(h, c)]
            nc.vector.tensor_mul(t[:sz, :], t[:sz, :], rec[h][:sz, :])
            nc.tensor.matmul(pout[:], th[c][:sz, h * 128:(h + 1) * 128],
                             t[:sz, :], start=(n_mm == 0),
                             stop=(n_mm == 4 * nch - 1))
            n_mm += 1
    nc.scalar.activation(out_xg[:], pout[:], AF.Relu, bias=g_b[:], scale=0.25)


def build_nc(imgs=64):
    nodes = NCORES * imgs
    nch = len(_chunks(nodes))
    nc = bacc.Bacc(None, target_bir_lowering=False, debug=False)

    # ---- per-core parameters -------------------------------------------
    x_in = nc.declare_dram_parameter("x", [imgs, CIN, HW], F32R, isOutput=False)
    conf_in = nc.declare_dram_parameter("conf", [imgs, HW], F32, isOutput=False)
    fe_wT = nc.declare_dram_parameter("fe_wT", [CIN, HID], F32R, isOutput=False)
    fe_b = nc.declare_dram_parameter("fe_b", [HID, 1], F32, isOutput=False)
    w1T = nc.declare_dram_parameter("w1T", [HID, 9, HID], F32R, isOutput=False)
    b1 = nc.declare_dram_parameter("b1", [HID, 1], F32, isOutput=False)
    w2T = nc.declare_dram_parameter("w2T", [HID, 9, HID], F32R, isOutput=False)
    b2 = nc.declare_dram_parameter("b2", [HID, 1], F32, isOutput=False)
    op_wT = nc.declare_dram_parameter("op_wT", [HID, CIN], F32R, isOutput=False)
    op_b2 = nc.declare_dram_parameter("op_b2", [HID, 1], F32, isOutput=False)
    g_lin_p = [nc.declare_dram_parameter(f"g{i}_lin", [HID, HEADS * HID], F32R,
                                         isOutput=False) for i in (1, 2)]
    g_asbc_p = [nc.declare_dram_parameter(f"g{i}_asbc", [1, HEADS * HID], F32,
                                          isOutput=False) for i in (1, 2)]
    g_att_p = [nc.declare_dram_parameter(f"g{i}_att", [HID, 2 * HEADS], F32R,
                                         isOutput=False) for i in (1, 2)]
    g_b_p = [nc.declare_dram_parameter(f"g{i}_b", [HID, 1], F32,
                                       isOutput=False) for i in (1, 2)]
    lc_in = nc.declare_dram_parameter("lcT", [nodes, imgs], F32, isOutput=False)
    ones_in = nc.declare_dram_parameter("ones", [HID, 1], F32R, isOutput=False)
    zeros_in = nc.declare_dram_parameter("zeros", [HID, CANV * CANV], F32R,
                                         isOutput=False)

    y_out = nc.declare_dram_parameter("y", [imgs, CIN, HW], F32, isOutput=True)

    # collective bounce buffers (internal DRAM)
    ag_in = [nc.dram_tensor(f"ag{i}_in", [HID, imgs], F32R) for i in (1, 2)]
    ag_out = [nc.dram_tensor(f"ag{i}_out", [NCORES * HID, imgs], F32R,
                             addr_space="Shared") for i in (1, 2)]

    with tile.TileContext(nc) as tc:
        with (
            tc.tile_pool(name="const", bufs=1) as cp,
            tc.tile_pool(name="persist", bufs=1) as pp,
        ):
            def cload(shape, dt, src, tag):
                t = cp.tile(shape, dt, tag=tag)
                nc.sync.dma_start(t[:], src[:])
                return t

            t_fe_wT = cload([CIN, HID], F32R, fe_wT, "fe_wT")
            t_fe_b = cload([HID, 1], F32, fe_b, "fe_b")
            t_w1T = cload([HID, 9, HID], F32R, w1T, "w1T")
            t_b1 = cload([HID, 1], F32, b1, "b1")
            t_w2T = cload([HID, 9, HID], F32R, w2T, "w2T")
            t_b2 = cload([HID, 1], F32, b2, "b2")
            t_op_wT = cload([HID, CIN], F32R, op_wT, "op_wT")
            t_op_b2 = cload([HID, 1], F32, op_b2, "op_b2")
            t_ones = cload([HID, 1], F32R, ones_in, "ones")
            t_g_lin = [cload([HID, HEADS * HID], F32R, g_lin_p[i], f"g{i}_lin")
                       for i in range(2)]
            t_g_asr = [cload([1, HEADS * HID], F32, g_asbc_p[i],
                             f"g{i}_asr") for i in range(2)]
            t_g_att = [cload([HID, 2 * HEADS], F32R, g_att_p[i], f"g{i}_att")
                       for i in range(2)]
            t_g_b = [cload([HID, 1], F32, g_b_p[i], f"g{i}_b")
                     for i in range(2)]
            t_lc = cp.tile([HID, nch * imgs], F32, tag="lcT", name="lcT")
            for c, (s0, sz) in enumerate(_chunks(nodes)):
                nc.sync.dma_start(t_lc[:sz, c * imgs:(c + 1) * imgs],
                                  lc_in[s0:s0 + sz, :])

            # persistent state
            y1_all = pp.tile([HID, (imgs // 2) * HW], F32, tag="y1", name="y1")
            xna2 = pp.tile([HID, 2 * imgs], F32, tag="xna2", name="xna2")
            # ---- phase 1: conv stack over image pairs ------------------
            with (
                tc.tile_pool(name="canv", bufs=1) as cvp,
                tc.tile_pool(name="io", bufs=2) as iop,
                tc.tile_pool(name="wk", bufs=2) as wp,
                tc.tile_pool(name="psA", bufs=2, space="PSUM") as psA,
                tc.tile_pool(name="ps1", bufs=2, space="PSUM") as ps1,
                tc.tile_pool(name="ps2", bufs=2, space="PSUM") as ps2,
                tc.tile_pool(name="psO", bufs=2, space="PSUM") as psO,
            ):
                canv1 = [cvp.tile([HID, CANV, CANV], F32R, tag=f"cv1_{b}",
                                  name=f"cv1_{b}") for b in range(NCBUF)]
                canv2 = [cvp.tile([HID, CANV, CANV], F32R, tag=f"cv2_{b}",
                                  name=f"cv2_{b}") for b in range(NCBUF)]
                for t in canv1 + canv2:
                    nc.sync.dma_start(
                        t[:].rearrange("p a b -> p (a b)"), zeros_in[:])
                for p in range(imgs // 2):
                    for par in range(2):
                        i = 2 * p + par
                        ximg = iop.tile([CIN, HW], F32R, tag="x", name="x")
                        nc.sync.dma_start(ximg[:], x_in[i, :, :])
                        tcf = iop.tile([1, HW], F32, tag="cf", name="cf")
                        nc.sync.dma_start(tcf[:], conf_in[i:i + 1, :])
                        cbc = wp.tile([HID, HW], F32, tag="cbc", name="cbc")
                        nc.gpsimd.partition_broadcast(cbc[:], tcf[:])
                        cv1 = canv1[i % NCBUF]
                        cv2 = canv2[i % NCBUF]
                        # feature encoder 1x1 + BN + relu, then * conf
                        for h in range(2):
                            pxe = psA.tile([HID, 512], F32, tag="pxe", name="pxe")
                            nc.tensor.matmul(
                                pxe[:], t_fe_wT[:],
                                ximg[:, h * 512:(h + 1) * 512],
                                start=True, stop=True)
                            nc.scalar.activation(pxe[:], pxe[:], AF.Relu,
                                                 bias=t_fe_b[:])
                            nc.vector.tensor_mul(
                                cv1[:, 1 + 16 * h:17 + 16 * h, 1:33],
                                pxe[:].rearrange("p (r c) -> p r c", c=32),
                                cbc[:, h * 512:(h + 1) * 512]
                                .rearrange("p (r c) -> p r c", c=32))
                        # spatial conv 1 (3x3) + BN + relu -> canvas2
                        for h in range(2):
                            pc1 = ps1.tile([HID, 512], F32, tag="pc1", name="pc1")
                            for j in range(9):
                                dy, dx = divmod(j, 3)
                                nc.tensor.matmul(
                                    pc1[:], t_w1T[:, j, :],
                                    cv1[:, 16 * h + dy:16 * h + dy + 16,
                                        dx:dx + 32],
                                    start=(j == 0), stop=(j == 8))
                            nc.scalar.activation(
                                cv2[:, 1 + 16 * h:17 + 16 * h, 1:33],
                                pc1[:].rearrange("p (r c) -> p r c", c=32),
                                AF.Relu, bias=t_b1[:])
                        # spatial conv 2 + BN + relu -> xp; pooled accum;
                        # output projector 1x1 into pair-shared psum
                        txp = wp.tile([HID, HW], F32R, tag="txp", name="txp")
                        for h in range(2):
                            pc2 = ps2.tile([HID, 512], F32, tag="pc2", name="pc2")
                            for j in range(9):
                                dy, dx = divmod(j, 3)
                                nc.tensor.matmul(
                                    pc2[:], t_w2T[:, j, :],
                                    cv2[:, 16 * h + dy:16 * h + dy + 16,
                                        dx:dx + 32],
                                    start=(j == 0), stop=(j == 8))
                            nc.scalar.activation(
                                txp[:, h * 512:(h + 1) * 512], pc2[:],
                                AF.Relu, bias=t_b2[:],
                                accum_out=xna2[:, 2 * i + h:2 * i + h + 1])
                            pop = psO.tile([CIN, 512], F32, tag="pop",
                                           name="pop")
                            nc.tensor.matmul(
                                pop[:], t_op_wT[:],
                                txp[:, h * 512:(h + 1) * 512],
                                start=True, stop=True)
                            dst = y1_all[par * CIN:(par + 1) * CIN,
                                         p * HW + h * 512:p * HW + (h + 1) * 512]
                            if par == 0:
                                nc.scalar.activation(dst, pop[:], AF.Identity,
                                                     bias=t_op_b2[0:CIN, :])
                            else:
                                ty1 = wp.tile([CIN, 512], F32, tag="ty1",
                                              name="ty1")
                                nc.scalar.activation(ty1[:], pop[:],
                                                     AF.Identity,
                                                     bias=t_op_b2[0:CIN, :])
                                nc.sync.dma_start(dst, ty1[:])

            # ---- phase 2: gather + 2 GAT layers + z --------------------
            with (
                tc.tile_pool(name="g_sb", bufs=1) as sb,
                tc.tile_pool(name="g_ps", bufs=1, space="PSUM") as ps,
            ):
                xn_own = sb.tile([HID, imgs], F32R, tag="xn_own", name="xn_own")
                with nc.allow_low_precision(reason="f32r matmul operand"):
                    nc.vector.tensor_reduce(
                        xn_own[:], xna2[:].rearrange("p (i h) -> p i h", h=2),
                        axis=AX.X, op=ALU.add)

                t_g_asbc = []
                for i_ in range(2):
                    tb = sb.tile([HID, HEADS * HID], F32,
                                 tag=f"g{i_}_asbc", name=f"g{i_}_asbc")
                    nc.gpsimd.partition_broadcast(tb[:], t_g_asr[i_][:])
                    t_g_asbc.append(tb)

                xts = []
                x_own_cur = xn_own
                for li in range(2):
                    nc.sync.dma_start(ag_in[li][:], x_own_cur[:])
                    nc.gpsimd.collective_compute(
                        "AllGather", ALU.bypass, ins=[ag_in[li][:]],
                        outs=[ag_out[li][:]],
                        replica_groups=[list(range(NCORES))])
                    xt = sb.tile([HID, nodes], F32R, tag=f"xt{li}", name=f"xt{li}")
                    nc.sync.dma_start(
                        xt[:].rearrange("f (r n) -> f r n", r=NCORES),
                        ag_out[li][:].rearrange("(r f) n -> f r n", r=NCORES))
                    xts.append(xt)
                    xg = sb.tile([HID, imgs], F32R, tag=f"xg{li}_own", name=f"xg{li}_own")
                    _gat_layer(nc, sb, ps, imgs=imgs, nodes=nodes,
                               xt_all=xt, x_own=x_own_cur,
                               g_lin=t_g_lin[li], att_s_bc=t_g_asbc[li],
                               att_cols=t_g_att[li], g_b=t_g_b[li],
                               lc_t=t_lc, ones_col=t_ones, out_xg=xg, li=li)
                    x_own_cur = xg

                # z[ch, node] = op_w @ xg2 (per-node channel bias)
                pz = ps.tile([CIN, imgs], F32, tag="pz", name="pz")
                nc.tensor.matmul(pz[:], t_op_wT[:], x_own_cur[:],
                                 start=True, stop=True)
                tz = sb.tile([CIN, imgs], F32, tag="tz", name="tz")
                nc.scalar.copy(tz[:], pz[:])
                # pair layout: partitions 0-63 even image z, 64-127 odd
                tz2 = sb.tile([HID, imgs // 2], F32, tag="tz2", name="tz2")
                nc.sync.dma_start(tz2[0:CIN, :], tz[:, 0::2])
                nc.sync.dma_start(tz2[CIN:HID, :], tz[:, 1::2])

                # ---- phase 3: out = y1 + z, stream to DRAM -------------
                with tc.tile_pool(name="out", bufs=4) as op_:
                    for p in range(imgs // 2):
                        for h in range(2):
                            to = op_.tile([HID, 512], F32, tag="to", name="to")
                            nc.vector.tensor_scalar_add(
                                to[:],
                                y1_all[:, p * HW + h * 512:
                                       p * HW + (h + 1) * 512],
                                tz2[:, p:p + 1])
                            nc.sync.dma_start(
                                y_out[2 * p:2 * p + 2, :,
                                      h * 512:(h + 1) * 512], to[:])

    nc.compile()
    return nc


def prep_inputs(inputs, imgs=64):
    """Fold BN/pooling constants, build per-core input maps."""
    nodes = NCORES * imgs
    f = np.float32
    g = lambda k: np.asarray(inputs[k])

    def bnfold(w, b, gm, beta):
        scale = (np.asarray(gm, np.float64) / np.sqrt(1.0 + BN_EPS))
        w_eff = np.asarray(w, np.float64) * scale[:, None, None, None]
        b_eff = np.asarray(b, np.float64) * scale + np.asarray(beta, np.float64)
        return w_eff.astype(f), b_eff.astype(f)

    fe_we, fe_be = bnfold(g("fe_w"), g("fe_b"), g("fe_g"), g("fe_beta"))
    w1e, b1e = bnfold(g("sp_w1"), g("sp_b1"), g("sp_g1"), g("sp_be1"))
    w2e, b2e = bnfold(g("sp_w2"), g("sp_b2"), g("sp_g2"), g("sp_be2"))
    op_w, op_b = g("op_w").astype(f), g("op_b").astype(f)

    common = {
        "fe_wT": np.ascontiguousarray(fe_we[:, :, 0, 0].T),
        "fe_b": fe_be[:, None],
        "w1T": np.ascontiguousarray(
            w1e.reshape(HID, HID, 9).transpose(1, 2, 0)),
        "b1": b1e[:, None],
        "w2T": np.ascontiguousarray(
            w2e.reshape(HID, HID, 9).transpose(1, 2, 0)),
        "b2": b2e[:, None],
        "op_wT": np.ascontiguousarray(op_w[:, :, 0, 0].T),
        "op_b2": np.concatenate([op_b, op_b])[:, None],
        "ones": np.ones((HID, 1), f),
        "zeros": np.zeros((HID, CANV * CANV), f),
    }
    for i in (1, 2):
        lin = g(f"g{i}_lin").astype(f)
        if i == 1:
            lin = lin / f(HW)          # fold pooling mean into GAT-1 lin
        a_s, a_d = g(f"g{i}_as").astype(f), g(f"g{i}_ad").astype(f)
        common[f"g{i}_lin"] = lin
        common[f"g{i}_asbc"] = a_s.reshape(1, HEADS * HID).copy()
        common[f"g{i}_att"] = np.concatenate([a_s.T, a_d.T], axis=1)
        common[f"g{i}_b"] = g(f"g{i}_b").astype(f)[:, None]

    src = np.asarray(g("edge_index")[0], np.int64)
    dst = np.asarray(g("edge_index")[1], np.int64)
    cnt = np.zeros((nodes, nodes), np.float64)       # [src, dst]
    np.add.at(cnt, (src, dst), 1.0)
    cnt[np.arange(nodes), np.arange(nodes)] += 1.0
    lc = np.where(cnt > 0, np.log(np.maximum(cnt, 1e-12)), BIG_NEG).astype(f)

    x = g("x").astype(f).reshape(nodes, CIN, HW)
    conf = g("confidence_maps").astype(f).reshape(nodes, HW)

    in_maps = []
    for c in range(NCORES):
        sl = slice(c * imgs, (c + 1) * imgs)
        m = dict(common)
        m["x"] = np.ascontiguousarray(x[sl])
        m["conf"] = np.ascontiguousarray(conf[sl])
        m["lcT"] = np.ascontiguousarray(lc[:, sl])
        in_maps.append(m)
    return in_maps


_NC = None


def _get_nc():
    global _NC
    if _NC is None:
        _NC = build_nc(64)
    return _NC


def kernel(**inputs):
    in_maps = prep_inputs(inputs, imgs=64)
    nc = _get_nc()
    res = run_bass_kernel_spmd(nc, in_maps, list(range(NCORES)))
    y = np.concatenate([r["y"] for r in res.results], axis=0)
    return np.ascontiguousarray(y.reshape(NCORES * 64, CIN, 32, 32),
                                dtype=np.float32)


if __name__ == "__main__":
    build_nc(64)
    print("build ok")



# revision 3
# speedup vs baseline: 1.1868x; 1.1868x over previous
"""Trainium2 Bass kernel for ConfidenceGNNFusion (nn_ConfidenceGNNFusion).

Pipeline (per the reference model):
  xe  = relu(BN(conv1x1(x))) * conf
  xp  = relu(BN(conv3x3(relu(BN(conv3x3(xe))))))
  xn  = mean_hw(xp)                         # (N, HID) node features
  xg  = relu(GAT2(relu(GAT1(xn))))          # two 4-head GAT layers
  out = conv1x1(xg[:, :, None, None] + xp)  # == conv1x1(xp) + op_w @ xg

Sharding: data-parallel over images/nodes (64 per core, 8 cores).

Phase 1 is software-pipelined across images with stage offsets
(load | fe | conv1 | conv2+pool | op-conv) so the tensor engine never
waits on the scalar/vector canvas writes of the same image — the
un-pipelined version stalls ~2.3us per image, which also re-arms the
HAM idle throttle (K=4/8 for 3.4us each image).

The pooled node features are AllGathered in two pieces: images 0..47
right after their conv2 finishes (hidden under remaining conv work) and
images 48..63 at the end (small, latency-only). Both GAT layers run
with dst nodes sharded per core; a third AllGather redistributes
layer-1 outputs.

GAT dense-mask form (exact softmax up to the shift invariance):
  exT[s, d] = exp(lrelu(a_s[s] + a_d[d]) + log(cnt[s, d]))
  out[d]    = (1/sum_s exT[s, d]) * sum_s exT[s, d] * h[s]
cnt = host-precomputed edge multiplicity (incl. self loops); absent
edges get log-count -60 -> exp underflows to ~1e-27.

All matmul operands are float32r (TF32-like) which streams at bf16
speed for moving dims >= 256. Partition broadcasts on the GAT critical
path are done as PE outer products (ones x row) instead of gpsimd.
"""
import numpy as np

import concourse.bass as bass  # noqa: F401  (kept for interactive use)
import concourse.mybir as mybir
import concourse.tile as tile
from concourse import bacc
from concourse.bass_utils import run_bass_kernel_spmd

F32 = mybir.dt.float32
F32R = mybir.dt.float32r
AF = mybir.ActivationFunctionType
ALU = mybir.AluOpType
AX = mybir.AxisListType

NCORES = 8
CIN = 64
HID = 128
HW = 1024            # 32*32
HEADS = 4
CANV = 34            # zero-padded canvas edge (32 + 2)
BIG_NEG = -60.0      # log-count for absent edges
BN_EPS = 1e-5
NA = 48              # images in the early AllGather group


def _chunks(n, step=128):
    return [(s, min(step, n - s)) for s in range(0, n, step)]


def _gat_layer(nc, sb, ps, *, imgs, nodes, xt_all, x_own, g_lin, att_s_bc,
               att_cols, g_b, lc_t, ones_col, ones_row, out_xg, li):
    """One dense GAT layer; dst nodes = this core's `imgs`.

    xt_all: [128 feat, nodes] f32r, features of ALL nodes (feat-major)
    x_own:  [128 feat, imgs] f32r, features of OWN nodes
    g_lin:  [128, 512] f32r lhsT (in_feat, heads*D)
    att_s_bc: [128, 512] att_s flat row replicated over partitions
    att_cols: [128, 8] f32r; cols 0-3 att_s per head, 4-7 att_d per head
    lc_t:   [128, nch*imgs] log-count^T (src chunk c at cols c*imgs..)
    ones_col: [128, 1] f32r; ones_row: [1, 128] f32r
    out_xg: [128, imgs] f32r output (feat-major, own nodes)
    """
    ch = _chunks(nodes)
    nch = len(ch)

    # 1) h node-major for all nodes: th[c] = [node chunk, heads*D]
    th = []
    for c, (s0, sz) in enumerate(ch):
        ph = ps.tile([128, 512], F32, tag="g_h", name="g_h", bufs=2)
        nc.tensor.matmul(ph[:sz, :], xt_all[:, s0:s0 + sz], g_lin[:],
                         start=True, stop=True)
        t = sb.tile([128, 512], F32R, tag=f"g_th{c}_{li}", name=f"g_th{c}_{li}")
        nc.scalar.copy(t[:sz, :], ph[:sz, :])
        th.append(t)

    # 2) a_s node-major: a_s[node, head] = sum_d h * att_s
    ta_s = sb.tile([128, 4 * nch], F32, tag=f"g_as_{li}", name=f"g_as_{li}")
    for c, (s0, sz) in enumerate(ch):
        tm = sb.tile([128, 512], F32, tag="g_astmp", name="g_astmp")
        nc.vector.tensor_mul(tm[:sz, :], th[c][:sz, :], att_s_bc[:sz, :])
        nc.vector.tensor_reduce(
            ta_s[:sz, c * 4:(c + 1) * 4],
            tm[:sz, :].rearrange("p (h d) -> p h d", h=HEADS),
            axis=AX.X, op=ALU.add)

    # 3) a_d of own nodes as a partition-broadcast row, per head:
    #    hT_own = lin_h^T @ x_own ; a_d_own = att_d_h^T @ hT_own
    #    broadcast via PE outer product (ones_row x a_d)
    a_d_bc = []
    for h in range(HEADS):
        pho = ps.tile([128, imgs], F32, tag="g_hto", name="g_hto")
        nc.tensor.matmul(pho[:], g_lin[:, h * 128:(h + 1) * 128], x_own[:],
                         start=True, stop=True)
        tho = sb.tile([128, imgs], F32R, tag="g_hto_s", name="g_hto_s")
        nc.scalar.copy(tho[:], pho[:])
        pad = ps.tile([1, imgs], F32, tag="g_adp", name="g_adp")
        nc.tensor.matmul(pad[:], att_cols[:, 4 + h:5 + h], tho[:],
                         start=True, stop=True)
        tad = sb.tile([1, imgs], F32R, tag="g_adr", name="g_adr")
        with nc.allow_low_precision(reason="f32r matmul operand"):
            nc.scalar.copy(tad[:], pad[:])
        pbc = ps.tile([128, imgs], F32, tag="g_bc", name="g_bc", bufs=2)
        nc.tensor.matmul(pbc[:], ones_row[:], tad[:], start=True, stop=True)
        tbc = sb.tile([128, imgs], F32, tag=f"g_adbc{h}_{li}", name=f"g_adbc{h}_{li}")
        nc.scalar.copy(tbc[:], pbc[:])
        a_d_bc.append(tbc)

    # 4) exT[src, dst] per (head, chunk) + per-head denominator
    ex = {}
    rec = []
    for h in range(HEADS):
        pden = ps.tile([1, imgs], F32, tag="g_den", name="g_den")
        for c, (s0, sz) in enumerate(ch):
            t = sb.tile([128, imgs], F32R, tag=f"g_ex{h}_{c}_{li}", name=f"g_ex{h}_{c}_{li}")
            # S^T = a_d[dst] + a_s[src]; lrelu; + logcnt; exp
            nc.vector.tensor_scalar(
                out=t[:sz, :], in0=a_d_bc[h][:sz, :],
                scalar1=ta_s[:sz, c * 4 + h:c * 4 + h + 1], scalar2=None,
                op0=ALU.add)
            nc.vector.scalar_tensor_tensor(
                t[:sz, :], t[:sz, :], 0.2, t[:sz, :],
                op0=ALU.mult, op1=ALU.max)
            nc.vector.tensor_add(t[:sz, :], t[:sz, :],
                                 lc_t[:sz, c * imgs:(c + 1) * imgs])
            nc.scalar.activation(t[:sz, :], t[:sz, :], AF.Exp)
            nc.tensor.matmul(pden[:], ones_col[:sz, :], t[:sz, :],
                             start=(c == 0), stop=(c == nch - 1))
            ex[(h, c)] = t
        trec = sb.tile([1, imgs], F32R, tag="g_rec", name="g_rec")
        with nc.allow_low_precision(reason="f32r matmul operand"):
            nc.vector.reciprocal(trec[:], pden[:])
        prb = ps.tile([128, imgs], F32, tag="g_bc", name="g_bc", bufs=2)
        nc.tensor.matmul(prb[:], ones_row[:], trec[:], start=True, stop=True)
        tbc = sb.tile([128, imgs], F32, tag=f"g_recbc{h}_{li}", name=f"g_recbc{h}_{li}")
        nc.scalar.copy(tbc[:], prb[:])
        rec.append(tbc)

    # 5) normalize exT, weighted sum over src into one psum, head-mean
    pout = ps.tile([128, imgs], F32, tag="g_out", name="g_out")
    n_mm = 0
    for h in range(HEADS):
        for c, (s0, sz) in enumerate(ch):
            t = ex[(h, c)]
            nc.vector.tensor_mul(t[:sz, :], t[:sz, :], rec[h][:sz, :])
            nc.tensor.matmul(pout[:], th[c][:sz, h * 128:(h + 1) * 128],
                             t[:sz, :], start=(n_mm == 0),
                             stop=(n_mm == 4 * nch - 1))
            n_mm += 1
    nc.scalar.activation(out_xg[:], pout[:], AF.Relu, bias=g_b[:], scale=0.25)


def build_nc(imgs=64):
    nodes = NCORES * imgs
    nch = len(_chunks(nodes))
    nb = imgs - NA
    nc = bacc.Bacc(None, target_bir_lowering=False, debug=False)

    # ---- per-core parameters -------------------------------------------
    x_in = nc.declare_dram_parameter("x", [imgs, CIN, HW], F32R, isOutput=False)
    conf_in = nc.declare_dram_parameter("conf", [imgs, HW], F32, isOutput=False)
    fe_wT = nc.declare_dram_parameter("fe_wT", [CIN, HID], F32R, isOutput=False)
    fe_b = nc.declare_dram_parameter("fe_b", [HID, 1], F32, isOutput=False)
    w1T = nc.declare_dram_parameter("w1T", [HID, 9, HID], F32R, isOutput=False)
    b1 = nc.declare_dram_parameter("b1", [HID, 1], F32, isOutput=False)
    w2T = nc.declare_dram_parameter("w2T", [HID, 9, HID], F32R, isOutput=False)
    b2 = nc.declare_dram_parameter("b2", [HID, 1], F32, isOutput=False)
    op_wT = nc.declare_dram_parameter("op_wT", [HID, CIN], F32R, isOutput=False)
    op_b2 = nc.declare_dram_parameter("op_b2", [HID, 1], F32, isOutput=False)
    g_lin_p = [nc.declare_dram_parameter(f"g{i}_lin", [HID, HEADS * HID], F32R,
                                         isOutput=False) for i in (1, 2)]
    g_asbc_p = [nc.declare_dram_parameter(f"g{i}_asbc", [1, HEADS * HID], F32,
                                          isOutput=False) for i in (1, 2)]
    g_att_p = [nc.declare_dram_parameter(f"g{i}_att", [HID, 2 * HEADS], F32R,
                                         isOutput=False) for i in (1, 2)]
    g_b_p = [nc.declare_dram_parameter(f"g{i}_b", [HID, 1], F32,
                                       isOutput=False) for i in (1, 2)]
    lc_in = nc.declare_dram_parameter("lcT", [nodes, imgs], F32, isOutput=False)
    ones_in = nc.declare_dram_parameter("ones", [HID, 1], F32R, isOutput=False)
    onesr_in = nc.declare_dram_parameter("onesr", [1, HID], F32R, isOutput=False)
    zeros_in = nc.declare_dram_parameter("zeros", [HID, 132], F32R,
                                         isOutput=False)

    y_out = nc.declare_dram_parameter("y", [imgs, CIN, HW], F32, isOutput=True)

    # collective bounce buffers (internal DRAM)
    agA_in = nc.dram_tensor("agA_in", [HID, NA], F32R)
    agA_out = nc.dram_tensor("agA_out", [NCORES * HID, NA], F32R,
                             addr_space="Shared")
    agB_in = nc.dram_tensor("agB_in", [HID, nb], F32R)
    agB_out = nc.dram_tensor("agB_out", [NCORES * HID, nb], F32R,
                             addr_space="Shared")
    ag2_in = nc.dram_tensor("ag2_in", [HID, imgs], F32R)
    ag2_out = nc.dram_tensor("ag2_out", [NCORES * HID, imgs], F32R,
                             addr_space="Shared")

    with tile.TileContext(nc) as tc:
        with (
            tc.tile_pool(name="const", bufs=1) as cp,
            tc.tile_pool(name="persist", bufs=1) as pp,
        ):
            def cload(shape, dt, src, tag):
                t = cp.tile(shape, dt, tag=tag, name=tag)
                nc.sync.dma_start(t[:], src[:])
                return t

            t_fe_wT = cload([CIN, HID], F32R, fe_wT, "fe_wT")
            t_fe_b = cload([HID, 1], F32, fe_b, "fe_b")
            t_w1T = cload([HID, 9, HID], F32R, w1T, "w1T")
            t_b1 = cload([HID, 1], F32, b1, "b1")
            t_w2T = cload([HID, 9, HID], F32R, w2T, "w2T")
            t_b2 = cload([HID, 1], F32, b2, "b2")
            t_op_wT = cload([HID, CIN], F32R, op_wT, "op_wT")
            t_op_b2 = cload([HID, 1], F32, op_b2, "op_b2")
            t_ones = cload([HID, 1], F32R, ones_in, "ones")
            t_onesr = cload([1, HID], F32R, onesr_in, "onesr")
            t_zeros = cload([HID, 132], F32R, zeros_in, "zeros")
            t_g_lin = [cload([HID, HEADS * HID], F32R, g_lin_p[i], f"g{i}_lin")
                       for i in range(2)]
            t_g_asr = [cload([1, HEADS * HID], F32, g_asbc_p[i],
                             f"g{i}_asr") for i in range(2)]
            t_g_att = [cload([HID, 2 * HEADS], F32R, g_att_p[i], f"g{i}_att")
                       for i in range(2)]
            t_g_b = [cload([HID, 1], F32, g_b_p[i], f"g{i}_b")
                     for i in range(2)]
            t_lc = cp.tile([HID, nch * imgs], F32, tag="lcT", name="lcT")
            for c, (s0, sz) in enumerate(_chunks(nodes)):
                nc.sync.dma_start(t_lc[:sz, c * imgs:(c + 1) * imgs],
                                  lc_in[s0:s0 + sz, :])

            # persistent state
            y1_all = pp.tile([HID, (imgs // 2) * HW], F32, tag="y1", name="y1")
            xna2 = pp.tile([HID, 2 * imgs], F32, tag="xna2", name="xna2")
            xn_own = pp.tile([HID, imgs], F32R, tag="xn_own", name="xn_own")

            # att_s broadcast rows (gpsimd; off the critical path — only
            # needed when the GAT starts ~700us later)
            t_g_asbc = []
            for i_ in range(2):
                tb = pp.tile([HID, HEADS * HID], F32,
                             tag=f"g{i_}_asbc", name=f"g{i_}_asbc")
                nc.gpsimd.partition_broadcast(tb[:], t_g_asr[i_][:])
                t_g_asbc.append(tb)

            # ---- phase 1: software-pipelined conv stack ----------------
            with (
                tc.tile_pool(name="canv", bufs=1) as cvp,
                tc.tile_pool(name="io", bufs=2) as iop,
                tc.tile_pool(name="wk", bufs=2) as wp,
                tc.tile_pool(name="psA", bufs=2, space="PSUM") as psA,
                tc.tile_pool(name="ps1", bufs=2, space="PSUM") as ps1,
                tc.tile_pool(name="ps2", bufs=2, space="PSUM") as ps2,
                tc.tile_pool(name="psO", bufs=2, space="PSUM") as psO,
            ):
                canv1 = [cvp.tile([HID, CANV, CANV], F32R, tag=f"cv1_{b}",
                                  name=f"cv1_{b}") for b in range(2)]
                canv2 = [cvp.tile([HID, CANV, CANV], F32R, tag=f"cv2_{b}",
                                  name=f"cv2_{b}") for b in range(2)]
                for t in canv1 + canv2:
                    # zero only the halo ring; interiors are overwritten
                    nc.sync.dma_start(t[:, 0:1, :], t_zeros[:, 0:34])
                    nc.sync.dma_start(t[:, 33:34, :], t_zeros[:, 34:68])
                    nc.sync.dma_start(t[:, 1:33, 0:1],
                                      t_zeros[:, 68:100, None])
                    nc.sync.dma_start(t[:, 1:33, 33:34],
                                      t_zeros[:, 100:132, None])

                ximgs = {}
                cbcs = {}
                txps = {}

                def st_load(i):
                    ximg = iop.tile([CIN, HW], F32R, tag="x", name="x")
                    nc.sync.dma_start(ximg[:], x_in[i, :, :])
                    tcf = iop.tile([1, HW], F32, tag="cf", name="cf")
                    nc.sync.dma_start(tcf[:], conf_in[i:i + 1, :])
                    cbc = wp.tile([HID, HW], F32, tag="cbc", name="cbc")
                    nc.gpsimd.partition_broadcast(cbc[:], tcf[:])
                    ximgs[i] = ximg
                    cbcs[i] = cbc

                def st_fe(i):
                    ximg = ximgs.pop(i)
                    cbc = cbcs.pop(i)
                    cv1 = canv1[i % 2]
                    for h in range(2):
                        pxe = psA.tile([HID, 512], F32, tag="pxe", name="pxe")
                        nc.tensor.matmul(
                            pxe[:], t_fe_wT[:],
                            ximg[:, h * 512:(h + 1) * 512],
                            start=True, stop=True)
                        nc.scalar.activation(pxe[:], pxe[:], AF.Relu,
                                             bias=t_fe_b[:])
                        nc.vector.tensor_mul(
                            cv1[:, 1 + 16 * h:17 + 16 * h, 1:33],
                            pxe[:].rearrange("p (r c) -> p r c", c=32),
                            cbc[:, h * 512:(h + 1) * 512]
                            .rearrange("p (r c) -> p r c", c=32))

                def st_c1(i):
                    cv1 = canv1[i % 2]
                    cv2 = canv2[i % 2]
                    for h in range(2):
                        pc1 = ps1.tile([HID, 512], F32, tag="pc1", name="pc1")
                        for j in range(9):
                            dy, dx = divmod(j, 3)
                            nc.tensor.matmul(
                                pc1[:], t_w1T[:, j, :],
                                cv1[:, 16 * h + dy:16 * h + dy + 16,
                                    dx:dx + 32],
                                start=(j == 0), stop=(j == 8))
                        nc.scalar.activation(
                            cv2[:, 1 + 16 * h:17 + 16 * h, 1:33],
                            pc1[:].rearrange("p (r c) -> p r c", c=32),
                            AF.Relu, bias=t_b1[:])

                def st_c2(i):
                    cv2 = canv2[i % 2]
                    txp = wp.tile([HID, HW], F32R, tag="txp", name="txp")
                    txps[i] = txp
                    for h in range(2):
                        pc2 = ps2.tile([HID, 512], F32, tag="pc2", name="pc2")
                        for j in range(9):
                            dy, dx = divmod(j, 3)
                            nc.tensor.matmul(
                                pc2[:], t_w2T[:, j, :],
                                cv2[:, 16 * h + dy:16 * h + dy + 16,
                                    dx:dx + 32],
                                start=(j == 0), stop=(j == 8))
                        nc.scalar.activation(
                            txp[:, h * 512:(h + 1) * 512], pc2[:],
                            AF.Relu, bias=t_b2[:],
                            accum_out=xna2[:, 2 * i + h:2 * i + h + 1])

                def st_op(i):
                    txp = txps.pop(i)
                    p, par = divmod(i, 2)
                    for h in range(2):
                        pop = psO.tile([CIN, 512], F32, tag="pop", name="pop")
                        nc.tensor.matmul(
                            pop[:], t_op_wT[:],
                            txp[:, h * 512:(h + 1) * 512],
                            start=True, stop=True)
                        dst = y1_all[par * CIN:(par + 1) * CIN,
                                     p * HW + h * 512:p * HW + (h + 1) * 512]
                        if par == 0:
                            nc.scalar.activation(dst, pop[:], AF.Identity,
                                                 bias=t_op_b2[0:CIN, :])
                        else:
                            ty1 = wp.tile([CIN, 512], F32, tag="ty1",
                                          name="ty1")
                            nc.scalar.activation(ty1[:], pop[:],
                                                 AF.Identity,
                                                 bias=t_op_b2[0:CIN, :])
                            nc.sync.dma_start(dst, ty1[:])

                for it in range(imgs + 4):
                    if it < imgs:
                        st_load(it)
                    if 0 <= it - 1 < imgs:
                        st_fe(it - 1)
                    if 0 <= it - 2 < imgs:
                        st_c1(it - 2)
                    if 0 <= it - 3 < imgs:
                        st_c2(it - 3)
                        if it - 3 == NA - 1:
                            # early AllGather of the first NA pooled nodes
                            with nc.allow_low_precision(reason="f32r"):
                                nc.vector.tensor_reduce(
                                    xn_own[:, 0:NA],
                                    xna2[:, 0:2 * NA]
                                    .rearrange("p (i h) -> p i h", h=2),
                                    axis=AX.X, op=ALU.add)
                            nc.sync.dma_start(agA_in[:], xn_own[:, 0:NA])
                            nc.gpsimd.collective_compute(
                                "AllGather", ALU.bypass, ins=[agA_in[:]],
                                outs=[agA_out[:]],
                                replica_groups=[list(range(NCORES))])
                    if 0 <= it - 4 < imgs:
                        st_op(it - 4)

                # tail AllGather for the last images
                with nc.allow_low_precision(reason="f32r"):
                    nc.vector.tensor_reduce(
                        xn_own[:, NA:imgs],
                        xna2[:, 2 * NA:2 * imgs]
                        .rearrange("p (i h) -> p i h", h=2),
                        axis=AX.X, op=ALU.add)
                nc.sync.dma_start(agB_in[:], xn_own[:, NA:imgs])
                nc.gpsimd.collective_compute(
                    "AllGather", ALU.bypass, ins=[agB_in[:]],
                    outs=[agB_out[:]],
                    replica_groups=[list(range(NCORES))])

            # ---- phase 2: gather + 2 GAT layers + z --------------------
            with (
                tc.tile_pool(name="g_sb", bufs=1) as sb,
                tc.tile_pool(name="g_ps", bufs=1, space="PSUM") as ps,
            ):
                xt = sb.tile([HID, nodes], F32R, tag="xt0", name="xt0")
                xt_v = xt[:].rearrange("f (r n) -> f r n", r=NCORES)
                nc.sync.dma_start(
                    xt_v[:, :, 0:NA],
                    agA_out[:].rearrange("(r f) n -> f r n", r=NCORES))
                nc.sync.dma_start(
                    xt_v[:, :, NA:imgs],
                    agB_out[:].rearrange("(r f) n -> f r n", r=NCORES))

                xg1 = sb.tile([HID, imgs], F32R, tag="xg1", name="xg1")
                _gat_layer(nc, sb, ps, imgs=imgs, nodes=nodes,
                           xt_all=xt, x_own=xn_own,
                           g_lin=t_g_lin[0], att_s_bc=t_g_asbc[0],
                           att_cols=t_g_att[0], g_b=t_g_b[0],
                           lc_t=t_lc, ones_col=t_ones, ones_row=t_onesr,
                           out_xg=xg1, li=0)

                nc.sync.dma_start(ag2_in[:], xg1[:])
                nc.gpsimd.collective_compute(
                    "AllGather", ALU.bypass, ins=[ag2_in[:]],
                    outs=[ag2_out[:]],
                    replica_groups=[list(range(NCORES))])
                xt2 = sb.tile([HID, nodes], F32R, tag="xt1", name="xt1")
                nc.sync.dma_start(
                    xt2[:].rearrange("f (r n) -> f r n", r=NCORES),
                    ag2_out[:].rearrange("(r f) n -> f r n", r=NCORES))
                xg2 = sb.tile([HID, imgs], F32R, tag="xg2", name="xg2")
                _gat_layer(nc, sb, ps, imgs=imgs, nodes=nodes,
                           xt_all=xt2, x_own=xg1,
                           g_lin=t_g_lin[1], att_s_bc=t_g_asbc[1],
                           att_cols=t_g_att[1], g_b=t_g_b[1],
                           lc_t=t_lc, ones_col=t_ones, ones_row=t_onesr,
                           out_xg=xg2, li=1)

                # z[ch, node] = op_w @ xg2 (per-node channel bias)
                pz = ps.tile([CIN, imgs], F32, tag="g_hto", name="pz")
                nc.tensor.matmul(pz[:], t_op_wT[:], xg2[:],
                                 start=True, stop=True)
                tz = sb.tile([CIN, imgs], F32, tag="tz", name="tz")
                nc.scalar.copy(tz[:], pz[:])
                # pair layout: partitions 0-63 even image z, 64-127 odd
                tz2 = sb.tile([HID, imgs // 2], F32, tag="tz2", name="tz2")
                nc.sync.dma_start(tz2[0:CIN, :], tz[:, 0::2])
                nc.sync.dma_start(tz2[CIN:HID, :], tz[:, 1::2])

                # ---- phase 3: out = y1 + z, stream to DRAM -------------
                with tc.tile_pool(name="out", bufs=6) as op_:
                    for p in range(imgs // 2):
                        for h in range(2):
                            to = op_.tile([HID, 512], F32, tag="to", name="to")
                            src = y1_all[:, p * HW + h * 512:
                                         p * HW + (h + 1) * 512]
                            if h == 0:
                                nc.scalar.activation(
                                    to[:], src, AF.Identity,
                                    bias=tz2[:, p:p + 1])
                            else:
                                nc.vector.tensor_scalar_add(
                                    to[:], src, tz2[:, p:p + 1])
                            nc.sync.dma_start(
                                y_out[2 * p:2 * p + 2, :,
                                      h * 512:(h + 1) * 512], to[:])

    nc.compile()
    return nc


def prep_inputs(inputs, imgs=64):
    """Fold BN/pooling constants, build per-core input maps."""
    nodes = NCORES * imgs
    f = np.float32
    g = lambda k: np.asarray(inputs[k])

    def bnfold(w, b, gm, beta):
        scale = (np.asarray(gm, np.float64) / np.sqrt(1.0 + BN_EPS))
        w_eff = np.asarray(w, np.float64) * scale[:, None, None, None]
        b_eff = np.asarray(b, np.float64) * scale + np.asarray(beta, np.float64)
        return w_eff.astype(f), b_eff.astype(f)

    fe_we, fe_be = bnfold(g("fe_w"), g("fe_b"), g("fe_g"), g("fe_beta"))
    w1e, b1e = bnfold(g("sp_w1"), g("sp_b1"), g("sp_g1"), g("sp_be1"))
    w2e, b2e = bnfold(g("sp_w2"), g("sp_b2"), g("sp_g2"), g("sp_be2"))
    op_w, op_b = g("op_w").astype(f), g("op_b").astype(f)

    common = {
        "fe_wT": np.ascontiguousarray(fe_we[:, :, 0, 0].T),
        "fe_b": fe_be[:, None],
        "w1T": np.ascontiguousarray(
            w1e.reshape(HID, HID, 9).transpose(1, 2, 0)),
        "b1": b1e[:, None],
        "w2T": np.ascontiguousarray(
            w2e.reshape(HID, HID, 9).transpose(1, 2, 0)),
        "b2": b2e[:, None],
        "op_wT": np.ascontiguousarray(op_w[:, :, 0, 0].T),
        "op_b2": np.concatenate([op_b, op_b])[:, None],
        "ones": np.ones((HID, 1), f),
        "onesr": np.ones((1, HID), f),
        "zeros": np.zeros((HID, 132), f),
    }
    for i in (1, 2):
        lin = g(f"g{i}_lin").astype(f)
        if i == 1:
            lin = lin / f(HW)          # fold pooling mean into GAT-1 lin
        a_s, a_d = g(f"g{i}_as").astype(f), g(f"g{i}_ad").astype(f)
        common[f"g{i}_lin"] = lin
        common[f"g{i}_asbc"] = a_s.reshape(1, HEADS * HID).copy()
        common[f"g{i}_att"] = np.concatenate([a_s.T, a_d.T], axis=1)
        common[f"g{i}_b"] = g(f"g{i}_b").astype(f)[:, None]

    src = np.asarray(g("edge_index")[0], np.int64)
    dst = np.asarray(g("edge_index")[1], np.int64)
    cnt = np.zeros((nodes, nodes), np.float64)       # [src, dst]
    np.add.at(cnt, (src, dst), 1.0)
    cnt[np.arange(nodes), np.arange(nodes)] += 1.0
    lc = np.where(cnt > 0, np.log(np.maximum(cnt, 1e-12)), BIG_NEG).astype(f)

    x = g("x").astype(f).reshape(nodes, CIN, HW)
    conf = g("confidence_maps").astype(f).reshape(nodes, HW)

    in_maps = []
    for c in range(NCORES):
        sl = slice(c * imgs, (c + 1) * imgs)
        m = dict(common)
        m["x"] = np.ascontiguousarray(x[sl])
        m["conf"] = np.ascontiguousarray(conf[sl])
        m["lcT"] = np.ascontiguousarray(lc[:, sl])
        in_maps.append(m)
    return in_maps


_NC = None


def _get_nc():
    global _NC
    if _NC is None:
        _NC = build_nc(64)
    return _NC


def kernel(**inputs):
    in_maps = prep_inputs(inputs, imgs=64)
    nc = _get_nc()
    res = run_bass_kernel_spmd(nc, in_maps, list(range(NCORES)))
    y = np.concatenate([r["y"] for r in res.results], axis=0)
    return np.ascontiguousarray(y.reshape(NCORES * 64, CIN, 32, 32),
                                dtype=np.float32)


if __name__ == "__main__":
    build_nc(64)
    print("build ok")


# revision 6
# speedup vs baseline: 1.2041x; 1.0145x over previous
"""Trainium2 Bass kernel for ConfidenceGNNFusion (nn_ConfidenceGNNFusion).

Pipeline (per the reference model):
  xe  = relu(BN(conv1x1(x))) * conf
  xp  = relu(BN(conv3x3(relu(BN(conv3x3(xe))))))
  xn  = mean_hw(xp)                         # (N, HID) node features
  xg  = relu(GAT2(relu(GAT1(xn))))          # two 4-head GAT layers
  out = conv1x1(xg[:, :, None, None] + xp)  # == conv1x1(xp) + op_w @ xg

Sharding: data-parallel over images/nodes (64 per core, 8 cores).

Phase 1 is software-pipelined across images with stage offsets
(load | fe | conv1 | conv2+pool | op-conv) so the tensor engine never
waits on the scalar/vector canvas writes of the same image — the
un-pipelined version stalls ~2.3us per image, which also re-arms the
HAM idle throttle (K=4/8 for 3.4us each image).

The pooled node features are AllGathered in two pieces: images 0..47
right after their conv2 finishes (hidden under remaining conv work) and
images 48..63 at the end (small, latency-only). Both GAT layers run
with dst nodes sharded per core; a third AllGather redistributes
layer-1 outputs.

GAT dense-mask form (exact softmax up to the shift invariance):
  exT[s, d] = exp(lrelu(a_s[s] + a_d[d]) + log(cnt[s, d]))
  out[d]    = (1/sum_s exT[s, d]) * sum_s exT[s, d] * h[s]
cnt = host-precomputed edge multiplicity (incl. self loops); absent
edges get log-count -60 -> exp underflows to ~1e-27.

All matmul operands are float32r (TF32-like) which streams at bf16
speed for moving dims >= 256. Partition broadcasts on the GAT critical
path are done as PE outer products (ones x row) instead of gpsimd.
"""
import numpy as np

import concourse.bass as bass  # noqa: F401  (kept for interactive use)
import concourse.mybir as mybir
import concourse.tile as tile
from concourse import bacc
from concourse.bass_utils import run_bass_kernel_spmd

F32 = mybir.dt.float32
F32R = mybir.dt.float32r
AF = mybir.ActivationFunctionType
ALU = mybir.AluOpType
AX = mybir.AxisListType

NCORES = 8
CIN = 64
HID = 128
HW = 1024            # 32*32
HEADS = 4
CANV = 34            # zero-padded canvas edge (32 + 2)
BIG_NEG = -60.0      # log-count for absent edges
BN_EPS = 1e-5
NA = 48              # images in the early AllGather group


def _chunks(n, step=128):
    return [(s, min(step, n - s)) for s in range(0, n, step)]


def _gat_layer(nc, sb, ps, *, imgs, nodes, xt_all, x_own, g_lin, att_s_bc,
               att_cols, g_b, lc_t, ones_col, ones_row, out_xg, li):
    """One dense GAT layer; dst nodes = this core's `imgs`.

    xt_all: [128 feat, nodes] f32r, features of ALL nodes (feat-major)
    x_own:  [128 feat, imgs] f32r, features of OWN nodes
    g_lin:  [128, 512] f32r lhsT (in_feat, heads*D)
    att_s_bc: [128, 512] att_s flat row replicated over partitions
    att_cols: [128, 8] f32r; cols 0-3 att_s per head, 4-7 att_d per head
    lc_t:   [128, nch*imgs] log-count^T (src chunk c at cols c*imgs..)
    ones_col: [128, 1] f32r; ones_row: [1, 128] f32r
    out_xg: [128, imgs] f32r output (feat-major, own nodes)
    """
    ch = _chunks(nodes)
    nch = len(ch)

    # 1) h node-major for all nodes: th[c] = [node chunk, heads*D]
    th = []
    for c, (s0, sz) in enumerate(ch):
        ph = ps.tile([128, 512], F32, tag="g_h", name="g_h", bufs=2)
        nc.tensor.matmul(ph[:sz, :], xt_all[:, s0:s0 + sz], g_lin[:],
                         start=True, stop=True)
        t = sb.tile([128, 512], F32R, tag=f"g_th{c}_{li}", name=f"g_th{c}_{li}")
        nc.scalar.copy(t[:sz, :], ph[:sz, :])
        th.append(t)

    # 2) a_s node-major: a_s[node, head] = sum_d h * att_s
    ta_s = sb.tile([128, 4 * nch], F32, tag=f"g_as_{li}", name=f"g_as_{li}")
    for c, (s0, sz) in enumerate(ch):
        tm = sb.tile([128, 512], F32, tag="g_astmp", name="g_astmp")
        nc.vector.tensor_mul(tm[:sz, :], th[c][:sz, :], att_s_bc[:sz, :])
        nc.vector.tensor_reduce(
            ta_s[:sz, c * 4:(c + 1) * 4],
            tm[:sz, :].rearrange("p (h d) -> p h d", h=HEADS),
            axis=AX.X, op=ALU.add)

    # 3) a_d of own nodes as a partition-broadcast row, per head:
    #    hT_own = lin_h^T @ x_own ; a_d_own = att_d_h^T @ hT_own
    #    broadcast via PE outer product (ones_row x a_d)
    a_d_bc = []
    for h in range(HEADS):
        pho = ps.tile([128, imgs], F32, tag="g_hto", name="g_hto")
        nc.tensor.matmul(pho[:], g_lin[:, h * 128:(h + 1) * 128], x_own[:],
                         start=True, stop=True)
        tho = sb.tile([128, imgs], F32R, tag="g_hto_s", name="g_hto_s")
        nc.scalar.copy(tho[:], pho[:])
        pad = ps.tile([1, imgs], F32, tag="g_adp", name="g_adp")
        nc.tensor.matmul(pad[:], att_cols[:, 4 + h:5 + h], tho[:],
                         start=True, stop=True)
        tad = sb.tile([1, imgs], F32R, tag="g_adr", name="g_adr")
        with nc.allow_low_precision(reason="f32r matmul operand"):
            nc.scalar.copy(tad[:], pad[:])
        pbc = ps.tile([128, imgs], F32, tag="g_bc", name="g_bc", bufs=2)
        nc.tensor.matmul(pbc[:], ones_row[:], tad[:], start=True, stop=True)
        tbc = sb.tile([128, imgs], F32, tag=f"g_adbc{h}_{li}", name=f"g_adbc{h}_{li}")
        nc.scalar.copy(tbc[:], pbc[:])
        a_d_bc.append(tbc)

    # 4) exT[src, dst] per (head, chunk) + per-head denominator
    ex = {}
    rec = []
    for h in range(HEADS):
        pden = ps.tile([1, imgs], F32, tag="g_den", name="g_den")
        for c, (s0, sz) in enumerate(ch):
            t = sb.tile([128, imgs], F32R, tag=f"g_ex{h}_{c}_{li}", name=f"g_ex{h}_{c}_{li}")
            # S^T = a_d[dst] + a_s[src]; lrelu; + logcnt; exp
            nc.vector.tensor_scalar(
                out=t[:sz, :], in0=a_d_bc[h][:sz, :],
                scalar1=ta_s[:sz, c * 4 + h:c * 4 + h + 1], scalar2=None,
                op0=ALU.add)
            nc.vector.scalar_tensor_tensor(
                t[:sz, :], t[:sz, :], 0.2, t[:sz, :],
                op0=ALU.mult, op1=ALU.max)
            nc.vector.tensor_add(t[:sz, :], t[:sz, :],
                                 lc_t[:sz, c * imgs:(c + 1) * imgs])
            nc.scalar.activation(t[:sz, :], t[:sz, :], AF.Exp)
            nc.tensor.matmul(pden[:], ones_col[:sz, :], t[:sz, :],
                             start=(c == 0), stop=(c == nch - 1))
            ex[(h, c)] = t
        trec = sb.tile([1, imgs], F32R, tag="g_rec", name="g_rec")
        with nc.allow_low_precision(reason="f32r matmul operand"):
            nc.vector.reciprocal(trec[:], pden[:])
        prb = ps.tile([128, imgs], F32, tag="g_bc", name="g_bc", bufs=2)
        nc.tensor.matmul(prb[:], ones_row[:], trec[:], start=True, stop=True)
        tbc = sb.tile([128, imgs], F32, tag=f"g_recbc{h}_{li}", name=f"g_recbc{h}_{li}")
        nc.scalar.copy(tbc[:], prb[:])
        rec.append(tbc)

    # 5) normalize exT, weighted sum over src into one psum, head-mean
    pout = ps.tile([128, imgs], F32, tag="g_out", name="g_out")
    n_mm = 0
    for h in range(HEADS):
        for c, (s0, sz) in enumerate(ch):
            t = ex[(h, c)]
            nc.vector.tensor_mul(t[:sz, :], t[:sz, :], rec[h][:sz, :])
            nc.tensor.matmul(pout[:], th[c][:sz, h * 128:(h + 1) * 128],
                             t[:sz, :], start=(n_mm == 0),
                             stop=(n_mm == 4 * nch - 1))
            n_mm += 1
    nc.scalar.activation(out_xg[:], pout[:], AF.Relu, bias=g_b[:], scale=0.25)


def build_nc(imgs=64):
    nodes = NCORES * imgs
    nch = len(_chunks(nodes))
    nb = imgs - NA
    nc = bacc.Bacc(None, target_bir_lowering=False, debug=False)

    # ---- per-core parameters -------------------------------------------
    x_in = nc.declare_dram_parameter("x", [imgs, CIN, HW], F32R, isOutput=False)
    conf_in = nc.declare_dram_parameter("conf", [imgs, HW], F32, isOutput=False)
    fe_wT = nc.declare_dram_parameter("fe_wT", [CIN, HID], F32R, isOutput=False)
    fe_b = nc.declare_dram_parameter("fe_b", [HID, 1], F32, isOutput=False)
    w1T = nc.declare_dram_parameter("w1T", [HID, 9, HID], F32R, isOutput=False)
    b1 = nc.declare_dram_parameter("b1", [HID, 1], F32, isOutput=False)
    w2T = nc.declare_dram_parameter("w2T", [HID, 9, HID], F32R, isOutput=False)
    b2 = nc.declare_dram_parameter("b2", [HID, 1], F32, isOutput=False)
    op_wT = nc.declare_dram_parameter("op_wT", [HID, CIN], F32R, isOutput=False)
    op_b2 = nc.declare_dram_parameter("op_b2", [HID, 1], F32, isOutput=False)
    g_lin_p = [nc.declare_dram_parameter(f"g{i}_lin", [HID, HEADS * HID], F32R,
                                         isOutput=False) for i in (1, 2)]
    g_asbc_p = [nc.declare_dram_parameter(f"g{i}_asbc", [1, HEADS * HID], F32,
                                          isOutput=False) for i in (1, 2)]
    g_att_p = [nc.declare_dram_parameter(f"g{i}_att", [HID, 2 * HEADS], F32R,
                                         isOutput=False) for i in (1, 2)]
    g_b_p = [nc.declare_dram_parameter(f"g{i}_b", [HID, 1], F32,
                                       isOutput=False) for i in (1, 2)]
    lc_in = nc.declare_dram_parameter("lcT", [nodes, imgs], F32, isOutput=False)
    ones_in = nc.declare_dram_parameter("ones", [HID, 1], F32R, isOutput=False)
    onesr_in = nc.declare_dram_parameter("onesr", [1, HID], F32R, isOutput=False)
    zeros_in = nc.declare_dram_parameter("zeros", [HID, 132], F32R,
                                         isOutput=False)

    y_out = nc.declare_dram_parameter("y", [imgs, CIN, HW], F32, isOutput=True)

    # collective bounce buffers (internal DRAM)
    agA_in = nc.dram_tensor("agA_in", [HID, NA], F32R)
    agA_out = nc.dram_tensor("agA_out", [NCORES * HID, NA], F32R,
                             addr_space="Shared")
    agB_in = nc.dram_tensor("agB_in", [HID, nb], F32R)
    agB_out = nc.dram_tensor("agB_out", [NCORES * HID, nb], F32R,
                             addr_space="Shared")
    ag2_in = nc.dram_tensor("ag2_in", [HID, imgs], F32R)
    ag2_out = nc.dram_tensor("ag2_out", [NCORES * HID, imgs], F32R,
                             addr_space="Shared")

    with tile.TileContext(nc) as tc:
        with (
            tc.tile_pool(name="const", bufs=1) as cp,
            tc.tile_pool(name="persist", bufs=1) as pp,
        ):
            def cload(shape, dt, src, tag):
                t = cp.tile(shape, dt, tag=tag, name=tag)
                nc.sync.dma_start(t[:], src[:])
                return t

            t_fe_wT = cload([CIN, HID], F32R, fe_wT, "fe_wT")
            t_fe_b = cload([HID, 1], F32, fe_b, "fe_b")
            t_w1T = cload([HID, 9, HID], F32R, w1T, "w1T")
            t_b1 = cload([HID, 1], F32, b1, "b1")
            t_w2T = cload([HID, 9, HID], F32R, w2T, "w2T")
            t_b2 = cload([HID, 1], F32, b2, "b2")
            t_op_wT = cload([HID, CIN], F32R, op_wT, "op_wT")
            t_op_b2 = cload([HID, 1], F32, op_b2, "op_b2")
            t_ones = cload([HID, 1], F32R, ones_in, "ones")
            t_onesr = cload([1, HID], F32R, onesr_in, "onesr")
            t_zeros = cload([HID, 132], F32R, zeros_in, "zeros")
            t_g_lin = [cload([HID, HEADS * HID], F32R, g_lin_p[i], f"g{i}_lin")
                       for i in range(2)]
            t_g_asr = [cload([1, HEADS * HID], F32, g_asbc_p[i],
                             f"g{i}_asr") for i in range(2)]
            t_g_att = [cload([HID, 2 * HEADS], F32R, g_att_p[i], f"g{i}_att")
                       for i in range(2)]
            t_g_b = [cload([HID, 1], F32, g_b_p[i], f"g{i}_b")
                     for i in range(2)]
            t_lc = cp.tile([HID, nch * imgs], F32, tag="lcT", name="lcT")
            for c, (s0, sz) in enumerate(_chunks(nodes)):
                nc.sync.dma_start(t_lc[:sz, c * imgs:(c + 1) * imgs],
                                  lc_in[s0:s0 + sz, :])

            # persistent state
            y1_all = pp.tile([HID, (imgs // 2) * HW], F32, tag="y1", name="y1")
            xna2 = pp.tile([HID, 2 * imgs], F32, tag="xna2", name="xna2")
            xn_own = pp.tile([HID, imgs], F32R, tag="xn_own", name="xn_own")

            # ---- phase 1: software-pipelined conv stack ----------------
            with (
                tc.tile_pool(name="canv", bufs=1) as cvp,
                tc.tile_pool(name="io", bufs=2) as iop,
                tc.tile_pool(name="wk", bufs=2) as wp,
                tc.tile_pool(name="psA", bufs=2, space="PSUM") as psA,
                tc.tile_pool(name="ps1", bufs=2, space="PSUM") as ps1,
                tc.tile_pool(name="ps2", bufs=2, space="PSUM") as ps2,
                tc.tile_pool(name="psO", bufs=2, space="PSUM") as psO,
            ):
                canv1 = [cvp.tile([HID, CANV, CANV], F32R, tag=f"cv1_{b}",
                                  name=f"cv1_{b}") for b in range(2)]
                canv2 = [cvp.tile([HID, CANV, CANV], F32R, tag=f"cv2_{b}",
                                  name=f"cv2_{b}") for b in range(2)]
                for t in canv1 + canv2:
                    # zero only the halo ring; interiors are overwritten
                    nc.sync.dma_start(t[:, 0:1, :], t_zeros[:, 0:34])
                    nc.sync.dma_start(t[:, 33:34, :], t_zeros[:, 34:68])
                    nc.sync.dma_start(t[:, 1:33, 0:1],
                                      t_zeros[:, 68:100, None])
                    nc.sync.dma_start(t[:, 1:33, 33:34],
                                      t_zeros[:, 100:132, None])

                ximgs = {}
                cbcs = {}
                txps = {}

                def st_load(i):
                    ximg = iop.tile([CIN, HW], F32R, tag="x", name="x")
                    nc.sync.dma_start(ximg[:], x_in[i, :, :])
                    tcf = iop.tile([1, HW], F32, tag="cf", name="cf")
                    nc.sync.dma_start(tcf[:], conf_in[i:i + 1, :])
                    cbc = wp.tile([HID, HW], F32, tag="cbc", name="cbc")
                    nc.gpsimd.partition_broadcast(cbc[:], tcf[:])
                    ximgs[i] = ximg
                    cbcs[i] = cbc

                def st_fe(i):
                    ximg = ximgs.pop(i)
                    cbc = cbcs.pop(i)
                    cv1 = canv1[i % 2]
                    for h in range(2):
                        pxe = psA.tile([HID, 512], F32, tag="pxe", name="pxe")
                        nc.tensor.matmul(
                            pxe[:], t_fe_wT[:],
                            ximg[:, h * 512:(h + 1) * 512],
                            start=True, stop=True)
                        nc.scalar.activation(pxe[:], pxe[:], AF.Relu,
                                             bias=t_fe_b[:])
                        nc.vector.tensor_mul(
                            cv1[:, 1 + 16 * h:17 + 16 * h, 1:33],
                            pxe[:].rearrange("p (r c) -> p r c", c=32),
                            cbc[:, h * 512:(h + 1) * 512]
                            .rearrange("p (r c) -> p r c", c=32))

                def st_c1(i):
                    cv1 = canv1[i % 2]
                    cv2 = canv2[i % 2]
                    for h in range(2):
                        pc1 = ps1.tile([HID, 512], F32, tag="pc1", name="pc1")
                        for j in range(9):
                            dy, dx = divmod(j, 3)
                            nc.tensor.matmul(
                                pc1[:], t_w1T[:, j, :],
                                cv1[:, 16 * h + dy:16 * h + dy + 16,
                                    dx:dx + 32],
                                start=(j == 0), stop=(j == 8))
                        nc.scalar.activation(
                            cv2[:, 1 + 16 * h:17 + 16 * h, 1:33],
                            pc1[:].rearrange("p (r c) -> p r c", c=32),
                            AF.Relu, bias=t_b1[:])

                def st_c2(i):
                    cv2 = canv2[i % 2]
                    txp = wp.tile([HID, HW], F32R, tag="txp", name="txp")
                    txps[i] = txp
                    for h in range(2):
                        pc2 = ps2.tile([HID, 512], F32, tag="pc2", name="pc2")
                        for j in range(9):
                            dy, dx = divmod(j, 3)
                            nc.tensor.matmul(
                                pc2[:], t_w2T[:, j, :],
                                cv2[:, 16 * h + dy:16 * h + dy + 16,
                                    dx:dx + 32],
                                start=(j == 0), stop=(j == 8))
                        nc.scalar.activation(
                            txp[:, h * 512:(h + 1) * 512], pc2[:],
                            AF.Relu, bias=t_b2[:],
                            accum_out=xna2[:, 2 * i + h:2 * i + h + 1])

                def st_op(i):
                    txp = txps.pop(i)
                    p, par = divmod(i, 2)
                    for h in range(2):
                        pop = psO.tile([CIN, 512], F32, tag="pop", name="pop")
                        nc.tensor.matmul(
                            pop[:], t_op_wT[:],
                            txp[:, h * 512:(h + 1) * 512],
                            start=True, stop=True)
                        dst = y1_all[par * CIN:(par + 1) * CIN,
                                     p * HW + h * 512:p * HW + (h + 1) * 512]
                        if par == 0:
                            nc.scalar.activation(dst, pop[:], AF.Identity,
                                                 bias=t_op_b2[0:CIN, :])
                        else:
                            ty1 = wp.tile([CIN, 512], F32, tag="ty1",
                                          name="ty1")
                            nc.scalar.activation(ty1[:], pop[:],
                                                 AF.Identity,
                                                 bias=t_op_b2[0:CIN, :])
                            nc.sync.dma_start(dst, ty1[:])

                for it in range(imgs + 4):
                    if it < imgs:
                        st_load(it)
                    if 0 <= it - 1 < imgs:
                        st_fe(it - 1)
                    if 0 <= it - 2 < imgs:
                        st_c1(it - 2)
                    if 0 <= it - 3 < imgs:
                        st_c2(it - 3)
                        if it - 3 == NA - 1:
                            # early AllGather of the first NA pooled nodes
                            with nc.allow_low_precision(reason="f32r"):
                                nc.vector.tensor_reduce(
                                    xn_own[:, 0:NA],
                                    xna2[:, 0:2 * NA]
                                    .rearrange("p (i h) -> p i h", h=2),
                                    axis=AX.X, op=ALU.add)
                            nc.sync.dma_start(agA_in[:], xn_own[:, 0:NA])
                            nc.gpsimd.collective_compute(
                                "AllGather", ALU.bypass, ins=[agA_in[:]],
                                outs=[agA_out[:]],
                                replica_groups=[list(range(NCORES))])
                    if 0 <= it - 4 < imgs:
                        st_op(it - 4)

                # tail AllGather for the last images
                with nc.allow_low_precision(reason="f32r"):
                    nc.vector.tensor_reduce(
                        xn_own[:, NA:imgs],
                        xna2[:, 2 * NA:2 * imgs]
                        .rearrange("p (i h) -> p i h", h=2),
                        axis=AX.X, op=ALU.add)
                nc.sync.dma_start(agB_in[:], xn_own[:, NA:imgs])
                nc.gpsimd.collective_compute(
                    "AllGather", ALU.bypass, ins=[agB_in[:]],
                    outs=[agB_out[:]],
                    replica_groups=[list(range(NCORES))])

            # ---- phase 2: gather + 2 GAT layers + z --------------------
            with (
                tc.tile_pool(name="g_sb", bufs=1) as sb,
                tc.tile_pool(name="g_ps", bufs=1, space="PSUM") as ps,
            ):
                # att_s broadcast rows (gpsimd is idle here)
                t_g_asbc = []
                for i_ in range(2):
                    tb = pp.tile([HID, HEADS * HID], F32,
                                 tag=f"g{i_}_asbc", name=f"g{i_}_asbc")
                    nc.gpsimd.partition_broadcast(tb[:], t_g_asr[i_][:])
                    t_g_asbc.append(tb)

                xt = sb.tile([HID, nodes], F32R, tag="xt0", name="xt0")
                xt_v = xt[:].rearrange("f (r n) -> f r n", r=NCORES)
                nc.sync.dma_start(
                    xt_v[:, :, 0:NA],
                    agA_out[:].rearrange("(r f) n -> f r n", r=NCORES))
                nc.sync.dma_start(
                    xt_v[:, :, NA:imgs],
                    agB_out[:].rearrange("(r f) n -> f r n", r=NCORES))

                xg1 = sb.tile([HID, imgs], F32R, tag="xg1", name="xg1")
                _gat_layer(nc, sb, ps, imgs=imgs, nodes=nodes,
                           xt_all=xt, x_own=xn_own,
                           g_lin=t_g_lin[0], att_s_bc=t_g_asbc[0],
                           att_cols=t_g_att[0], g_b=t_g_b[0],
                           lc_t=t_lc, ones_col=t_ones, ones_row=t_onesr,
                           out_xg=xg1, li=0)

                nc.sync.dma_start(ag2_in[:], xg1[:])
                nc.gpsimd.collective_compute(
                    "AllGather", ALU.bypass, ins=[ag2_in[:]],
                    outs=[ag2_out[:]],
                    replica_groups=[list(range(NCORES))])
                xt2 = sb.tile([HID, nodes], F32R, tag="xt1", name="xt1")
                nc.sync.dma_start(
                    xt2[:].rearrange("f (r n) -> f r n", r=NCORES),
                    ag2_out[:].rearrange("(r f) n -> f r n", r=NCORES))
                xg2 = sb.tile([HID, imgs], F32R, tag="xg2", name="xg2")
                _gat_layer(nc, sb, ps, imgs=imgs, nodes=nodes,
                           xt_all=xt2, x_own=xg1,
                           g_lin=t_g_lin[1], att_s_bc=t_g_asbc[1],
                           att_cols=t_g_att[1], g_b=t_g_b[1],
                           lc_t=t_lc, ones_col=t_ones, ones_row=t_onesr,
                           out_xg=xg2, li=1)

                # z[ch, node] = op_w @ xg2 (per-node channel bias)
                pz = ps.tile([CIN, imgs], F32, tag="g_hto", name="pz")
                nc.tensor.matmul(pz[:], t_op_wT[:], xg2[:],
                                 start=True, stop=True)
                tz = sb.tile([CIN, imgs], F32, tag="tz", name="tz")
                nc.scalar.copy(tz[:], pz[:])
                # pair layout: partitions 0-63 even image z, 64-127 odd
                tz2 = sb.tile([HID, imgs // 2], F32, tag="tz2", name="tz2")
                nc.sync.dma_start(tz2[0:CIN, :], tz[:, 0::2])
                nc.sync.dma_start(tz2[CIN:HID, :], tz[:, 1::2])

                # ---- phase 3: out = y1 + z, stream to DRAM -------------
                with tc.tile_pool(name="out", bufs=4) as op_:
                    for p in range(imgs // 2):
                        to = op_.tile([HID, HW], F32, tag="to", name="to")
                        srcv = y1_all[:, p * HW:(p + 1) * HW]
                        if p % 2 == 0:
                            nc.scalar.activation(
                                to[:], srcv, AF.Identity,
                                bias=tz2[:, p:p + 1])
                        else:
                            nc.vector.tensor_scalar_add(
                                to[:], srcv, tz2[:, p:p + 1])
                        nc.sync.dma_start(
                            y_out[2 * p:2 * p + 2, :, :], to[:])

    nc.compile()
    return nc


def prep_inputs(inputs, imgs=64):
    """Fold BN/pooling constants, build per-core input maps."""
    nodes = NCORES * imgs
    f = np.float32
    g = lambda k: np.asarray(inputs[k])

    def bnfold(w, b, gm, beta):
        scale = (np.asarray(gm, np.float64) / np.sqrt(1.0 + BN_EPS))
        w_eff = np.asarray(w, np.float64) * scale[:, None, None, None]
        b_eff = np.asarray(b, np.float64) * scale + np.asarray(beta, np.float64)
        return w_eff.astype(f), b_eff.astype(f)

    fe_we, fe_be = bnfold(g("fe_w"), g("fe_b"), g("fe_g"), g("fe_beta"))
    w1e, b1e = bnfold(g("sp_w1"), g("sp_b1"), g("sp_g1"), g("sp_be1"))
    w2e, b2e = bnfold(g("sp_w2"), g("sp_b2"), g("sp_g2"), g("sp_be2"))
    op_w, op_b = g("op_w").astype(f), g("op_b").astype(f)

    common = {
        "fe_wT": np.ascontiguousarray(fe_we[:, :, 0, 0].T),
        "fe_b": fe_be[:, None],
        "w1T": np.ascontiguousarray(
            w1e.reshape(HID, HID, 9).transpose(1, 2, 0)),
        "b1": b1e[:, None],
        "w2T": np.ascontiguousarray(
            w2e.reshape(HID, HID, 9).transpose(1, 2, 0)),
        "b2": b2e[:, None],
        "op_wT": np.ascontiguousarray(op_w[:, :, 0, 0].T),
        "op_b2": np.concatenate([op_b, op_b])[:, None],
        "ones": np.ones((HID, 1), f),
        "onesr": np.ones((1, HID), f),
        "zeros": np.zeros((HID, 132), f),
    }
    for i in (1, 2):
        lin = g(f"g{i}_lin").astype(f)
        if i == 1:
            lin = lin / f(HW)          # fold pooling mean into GAT-1 lin
        a_s, a_d = g(f"g{i}_as").astype(f), g(f"g{i}_ad").astype(f)
        common[f"g{i}_lin"] = lin
        common[f"g{i}_asbc"] = a_s.reshape(1, HEADS * HID).copy()
        common[f"g{i}_att"] = np.concatenate([a_s.T, a_d.T], axis=1)
        common[f"g{i}_b"] = g(f"g{i}_b").astype(f)[:, None]

    src = np.asarray(g("edge_index")[0], np.int64)
    dst = np.asarray(g("edge_index")[1], np.int64)
    cnt = np.zeros((nodes, nodes), np.float64)       # [src, dst]
    np.add.at(cnt, (src, dst), 1.0)
    cnt[np.arange(nodes), np.arange(nodes)] += 1.0
    lc = np.where(cnt > 0, np.log(np.maximum(cnt, 1e-12)), BIG_NEG).astype(f)

    x = g("x").astype(f).reshape(nodes, CIN, HW)
    conf = g("confidence_maps").astype(f).reshape(nodes, HW)

    in_maps = []
    for c in range(NCORES):
        sl = slice(c * imgs, (c + 1) * imgs)
        m = dict(common)
        m["x"] = np.ascontiguousarray(x[sl])
        m["conf"] = np.ascontiguousarray(conf[sl])
        m["lcT"] = np.ascontiguousarray(lc[:, sl])
        in_maps.append(m)
    return in_maps


_NC = None


def _get_nc():
    global _NC
    if _NC is None:
        _NC = build_nc(64)
    return _NC


def kernel(**inputs):
    in_maps = prep_inputs(inputs, imgs=64)
    nc = _get_nc()
    res = run_bass_kernel_spmd(nc, in_maps, list(range(NCORES)))
    y = np.concatenate([r["y"] for r in res.results], axis=0)
    return np.ascontiguousarray(y.reshape(NCORES * 64, CIN, 32, 32),
                                dtype=np.float32)


if __name__ == "__main__":
    build_nc(64)
    print("build ok")
